# revision 13
# baseline (speedup 1.0000x reference)
"""CapsNet dynamic-routing kernel for 8 Trainium2 NeuronCores.

Sharding: tensor-parallel over N_OUT (8 output capsules per core). x_hat
(B, N_OUT, N_IN, D_OUT) is never materialized; every contraction over it is
re-expressed against W and x as PE matmuls:

  s_t[b,o,do]     = sum_{i,di} W[o,i,do,di] * c_t[b,o,i] * x[b,i,di]
  beta_inc[b,o,i] = sum_di ( sum_do v[b,o,do] W[o,i,do,di] ) * x[b,i,di]

vs the previous revision, this version:
  - splits input DMA per i-chunk so iteration-0 matmuls overlap the loads,
    and transfers wdo compactly (no zero rows);
  - drains the Wv PSUM chunks on three engines: DVE fused
    scalar_tensor_tensor (drain*f*x in one op), ACT copy + DVE mul, and
    ACT copy + GPSIMD mul;
  - kicks the softmax-Z AllReduce (bf16) immediately after exp and runs the
    e-transposes (h-packed 128x128) during the collective;
  - keeps the o-sharded softmax: one AllReduce per routing iteration
    (iters 1, 2), iteration-0 softmax folded into the squash scale.
"""

import os
import sys
import types

import numpy as np
import ml_dtypes

B = 64
N_IN = 1152
D_IN = 8
N_OUT = 64
D_OUT = 16
O_LOC = 8
N_CORES = 8
KD = N_IN * D_IN  # 9216
NCH = 9           # i chunks of 128
NJ = KD // 512    # 18 Wv psum chunks per group
EPS = 1e-8

bf16 = ml_dtypes.bfloat16

_CACHE = {}
last_exec_ns = None


def _install_ntff_hook():
    try:
        import antenv
    except ImportError:
        return
    if "antenv.axon_hooks" in sys.modules:
        return
    mod = types.ModuleType("antenv.axon_hooks")
    _state = {"hook": None}
    mod.set_axon_ntff_profile_hook = lambda h: _state.__setitem__("hook", h)
    mod.get_axon_ntff_profile_hook = lambda: _state["hook"]
    sys.modules["antenv.axon_hooks"] = mod
    antenv.axon_hooks = mod
    try:
        from trn_agent_boot.trn_boot import _ntff_profile_via_ctypes
        hook = _ntff_profile_via_ctypes("/opt/axon/libaxon_pjrt.so")
        if hook is not None:
            mod.set_axon_ntff_profile_hook(hook)
    except Exception:
        pass


def _build():
    import concourse.bacc as bacc
    import concourse.tile as tile
    import concourse.mybir as mybir

    dt = mybir.dt
    Alu = mybir.AluOpType
    Act = mybir.ActivationFunctionType

    nc = bacc.Bacc("TRN2", target_bir_lowering=False, debug=False,
                   num_devices=N_CORES)

    # ---- DRAM I/O ----
    d_xt = nc.dram_tensor("xt", [128, NCH, D_IN, B], dt.bfloat16,
                          kind="ExternalInput")
    d_wf = nc.dram_tensor("wf", [128, NCH, D_IN, O_LOC * D_OUT], dt.bfloat16,
                          kind="ExternalInput")
    d_wdo = nc.dram_tensor("wdo", [4, 16, 2, KD], dt.bfloat16,
                           kind="ExternalInput")
    d_xb = nc.dram_tensor("xb", [B, KD], dt.bfloat16, kind="ExternalInput")
    d_sel = nc.dram_tensor("sel", [128, B], dt.bfloat16, kind="ExternalInput")
    d_ones = nc.dram_tensor("onesbd", [128, 4], dt.float32,
                            kind="ExternalInput")
    d_idf = nc.dram_tensor("identf", [128, 128], dt.float32,
                           kind="ExternalInput")
    d_idb = nc.dram_tensor("identb", [128, 128], dt.bfloat16,
                           kind="ExternalInput")
    d_out = nc.dram_tensor("yout", [B, 2, 128], dt.float32,
                           kind="ExternalOutput")

    with tile.TileContext(nc) as tc:
        with (
            tc.tile_pool(name="const", bufs=1) as constp,
            tc.tile_pool(name="big", bufs=1) as bigp,
            tc.tile_pool(name="small", bufs=2) as smallp,
            tc.tile_pool(name="ps", bufs=1, space="PSUM") as psp,
            tc.tile_pool(name="psS", bufs=1, space="PSUM") as psS,
            tc.tile_pool(name="dram", bufs=1, space="DRAM") as dramp,
        ):
            # ---------- constants / inputs ----------
            sel = constp.tile([128, B], dt.bfloat16)
            nc.sync.dma_start(sel[:], d_sel[:])
            onesbd = constp.tile([128, 4], dt.float32)
            nc.sync.dma_start(onesbd[:], d_ones[:])
            idf = constp.tile([128, 128], dt.float32)
            nc.sync.dma_start(idf[:], d_idf[:])
            idb = constp.tile([128, 128], dt.bfloat16)
            nc.sync.dma_start(idb[:], d_idb[:])

            # split per i-chunk so iteration-0 matmuls start while the
            # remaining chunks stream in
            xt = constp.tile([128, NCH, D_IN, B], dt.bfloat16)
            wf = constp.tile([128, NCH, D_IN, O_LOC * D_OUT], dt.bfloat16)
            for t3 in range(3):
                tsl = slice(3 * t3, 3 * t3 + 3)
                nc.sync.dma_start(xt[:, tsl, :, :], d_xt[:, tsl, :, :])
                nc.sync.dma_start(wf[:, tsl, :, :], d_wf[:, tsl, :, :])
            # wdo / xb2 are only needed from iteration 1 on
            wdo = constp.tile([128, 2, KD], dt.bfloat16)
            for g in range(4):
                nc.sync.dma_start(wdo[32 * g:32 * g + 16, :, :], d_wdo[g])
            xb2 = constp.tile([128, KD], dt.bfloat16)
            nc.sync.dma_start(xb2[:B, :], d_xb[:])
            nc.sync.dma_start(xb2[B:, :], xb2[:B, :])

            # persistent state
            beta = [bigp.tile([128, N_IN], dt.float32, tag=f"beta{g}",
                              name=f"beta{g}")
                    for g in range(4)]
            sT = bigp.tile([128, 2, B], dt.bfloat16, tag="sT")
            sTf = bigp.tile([128, 2, B], dt.float32, tag="sTf")
            fT2 = bigp.tile([128, 4], dt.float32, tag="fT2")
            epst = bigp.tile([4, 1], dt.float32, tag="epst")
            nc.gpsimd.memset(epst[:], EPS)

            def squash_from_sTf(first):
                # sT (bf16) + sq from the already-materialized sTf
                sq = smallp.tile([128, 2, B], dt.float32, tag="sq")
                scale = (1.0 / N_OUT) if first else 1.0
                nc.vector.tensor_copy(sT[:], sTf[:])
                nc.scalar.activation(sq[:], sTf[:], Act.Square, scale=scale)
                n2ps = psp.tile([4, 2, B], dt.float32, tag="tiny")
                for h in range(2):
                    nc.tensor.matmul(n2ps[:, h, :], onesbd[:], sq[:, h, :],
                                     start=True, stop=True)
                n2 = smallp.tile([4, 2, B], dt.float32, tag="n2s")
                nc.vector.tensor_copy(n2[:], n2ps[:])
                a = smallp.tile([4, 2, B], dt.float32, tag="fa")
                srt = smallp.tile([4, 2, B], dt.float32, tag="fs")
                nc.scalar.activation(a[:], n2[:], Act.Copy, bias=1.0)
                nc.scalar.activation(srt[:], n2[:], Act.Sqrt, bias=epst[:])
                nc.vector.tensor_mul(a[:], a[:], srt[:])
                nc.vector.reciprocal(srt[:], a[:])
                nc.vector.tensor_mul(a[:], n2[:], srt[:])
                if first:
                    nc.vector.tensor_scalar_mul(a[:], a[:], 1.0 / N_OUT)
                fps = psp.tile([128, 4], dt.float32, tag="tiny")
                nc.tensor.transpose(
                    fps[:], a.rearrange("g h b -> g (h b)"), idf[:4, :4])
                nc.vector.tensor_copy(fT2[:], fps[:])

            # ---------- iteration 0 ----------
            # c0 is uniform, so s0 = (1/64) * sum_i x_hat: rhs is xt itself.
            # ic-outer loop order lets chunk ic start right after its DMA.
            n_acc = NCH * D_IN
            quarters = [(0, 1, 2), (3, 4, 5), (6, 7, 8)]
            qtags = [("sps", psS), ("eT", psp), ("wv", psp)]
            qtiles = []
            for qi, ics in enumerate(quarters):
                tag, pool = qtags[qi]
                qt = pool.tile([128, 2, B], dt.float32, tag=tag,
                               name=f"q0_{qi}",
                               bufs=1 if tag == "sps" else 2)
                qtiles.append(qt)
                nq = len(ics) * D_IN
                for o in range(O_LOC):
                    g, h = o % 4, o // 4
                    k = 0
                    for ic in ics:
                        for di in range(D_IN):
                            nc.tensor.matmul(
                                qt[32 * g:32 * g + 16, h, :],
                                wf[:, ic, di, 16 * o:16 * o + 16],
                                xt[:, ic, di, :],
                                start=(k == 0), stop=(k == nq - 1),
                                tile_position=(0, 32 * g),
                            )
                            k += 1
            nc.vector.tensor_copy(sTf[:], qtiles[0][:])
            for qi in (1, 2):
                nc.vector.tensor_add(sTf[:], sTf[:], qtiles[qi][:])
            squash_from_sTf(first=True)

            zin = dramp.tile([B, N_IN], dt.bfloat16)
            zout = dramp.tile([B, N_IN], dt.bfloat16)

            for it in (1, 2):
                # ----- beta increment -----
                # Wv matmuls (K=16, 8-way PE tiling), then drain+(*f)+(*x)
                # split across DVE (fused STT), ACT+DVE, ACT+GPSIMD.
                e = [bigp.tile([128, N_IN], dt.bfloat16, tag=f"e{g}",
                               name=f"e{g}_{it}")
                     for g in range(4)]
                for g in range(4):
                    pbuf = bigp.tile([128, KD], dt.bfloat16, tag="p",
                                     bufs=2)
                    for j in range(NJ):
                        wps = psp.tile([128, 512], dt.float32, tag="wv",
                                       bufs=2)
                        for h in range(2):
                            nc.tensor.matmul(
                                wps[64 * h:64 * h + 64, :],
                                sT[32 * g:32 * g + 16, h, :],
                                wdo[32 * g:32 * g + 16, h,
                                    512 * j:512 * (j + 1)],
                                start=True, stop=True,
                                tile_position=(32 * g, 64 * h),
                            )
                        sl = slice(512 * j, 512 * (j + 1))
                        if j in (9,):
                            nc.vector.tensor_scalar_mul(
                                pbuf[:, sl], wps[:], fT2[:, g:g + 1])
                        else:
                            nc.scalar.activation(
                                pbuf[:, sl], wps[:],
                                Act.Copy, scale=fT2[:, g:g + 1])
                        nc.vector.tensor_mul(pbuf[:, sl], pbuf[:, sl],
                                              xb2[:, sl])
                    pv = pbuf.rearrange("p (a two n) -> p a two n",
                                        a=4, two=2)
                    # q1 written in-place into pbuf's low half (the write
                    # trails the reads in stream order, so no clobber)
                    q1 = pbuf[:, :4 * N_IN].rearrange("p (a n) -> p a n",
                                                      a=4)
                    q1v = pbuf[:, :4 * N_IN].rearrange(
                        "p (a two n) -> p a two n", a=2, two=2)
                    q2 = bigp.tile([128, 2, N_IN], dt.bfloat16, tag="q2")
                    binc = bigp.tile([128, N_IN], dt.float32, tag="binc")
                    for st in range(3):
                        ssl = slice(384 * st, 384 * (st + 1))
                        nc.vector.tensor_add(q1[:, :, ssl],
                                             pv[:, :, 0, ssl],
                                             pv[:, :, 1, ssl])
                        nc.vector.tensor_add(q2[:, :, ssl],
                                             q1v[:, :, 0, ssl],
                                             q1v[:, :, 1, ssl])
                        if it == 1:
                            nc.vector.tensor_add(beta[g][:, ssl],
                                                 q2[:, 0, ssl],
                                                 q2[:, 1, ssl])
                        else:
                            nc.vector.tensor_add(binc[:, ssl],
                                                 q2[:, 0, ssl],
                                                 q2[:, 1, ssl])
                            nc.vector.tensor_add(beta[g][:, ssl],
                                                 beta[g][:, ssl],
                                                 binc[:, ssl])
                        nc.scalar.activation(e[g][:, ssl],
                                             beta[g][:, ssl], Act.Exp)

                # ----- softmax Z partials + AllReduce (kicked ASAP) -----
                zpart = bigp.tile([B, N_IN], dt.bfloat16, tag="zpart")
                for zc in range(3):
                    zps = psp.tile([B, 384], dt.float32, tag="z",
                                   name=f"zps{it}_{zc}", bufs=2)
                    for g in range(4):
                        nc.tensor.matmul(
                            zps[:], sel[:],
                            e[g][:, 384 * zc:384 * (zc + 1)],
                            start=(g == 0), stop=(g == 3),
                        )
                    nc.scalar.activation(
                        zpart[:, 384 * zc:384 * (zc + 1)], zps[:],
                        Act.Copy)
                nc.sync.dma_start(zin[:], zpart[:])
                nc.gpsimd.collective_compute(
                    "AllReduce", Alu.add,
                    ins=[zin.opt()], outs=[zout.opt()],
                    replica_groups=[list(range(N_CORES))],
                )
                zsbb = bigp.tile([B, N_IN], dt.bfloat16, tag="zsbb")
                nc.sync.dma_start(zsbb[:], zout[:])

                # ----- e transposes: run during the AllReduce -----
                # h-packed: one [128,128] transpose covers both o's of a group
                eTs = [bigp.tile([128, NCH, 2, B], dt.bfloat16, tag=f"eT{g}",
                                 name=f"eTs{g}_{it}")
                       for g in range(4)]
                for g in range(4):
                    for ic in range(NCH):
                        eTp = psp.tile([128, 128], dt.bfloat16, tag="eT",
                                       bufs=2, name=f"eTp{it}_{g}_{ic}")
                        nc.tensor.transpose(
                            eTp[:],
                            e[g][:, 128 * ic:128 * (ic + 1)],
                            idb[:])
                        nc.scalar.activation(
                            eTs[g][:, ic, :, :].rearrange("p h b -> p (h b)"),
                            eTp[:], Act.Copy)

                # ----- post-AllReduce: rz, rzT, then per-o c/y/s -----
                zsb = bigp.tile([B, N_IN], dt.float32, tag="zsb")
                rz = bigp.tile([B, N_IN], dt.float32, tag="rz")
                rzT = bigp.tile([128, NCH, B], dt.bfloat16, tag="rzT")
                for ic in range(NCH):
                    blk = slice(128 * ic, 128 * (ic + 1))
                    nc.vector.tensor_copy(zsb[:, blk], zsbb[:, blk])
                    nc.vector.reciprocal_approx_fast(rz[:, blk], zsb[:, blk])
                    rzp = psp.tile([128, B], dt.float32, tag="eT",
                                   bufs=2, name=f"rzp{it}_{ic}")
                    nc.tensor.transpose(rzp[:], rz[:, blk], idf[:64, :64])
                    nc.scalar.activation(rzT[:, ic, :], rzp[:], Act.Copy)

                psY = psS.tile([128, 2, B], dt.float32, tag="sps")
                for o in range(O_LOC):
                    g, h = o % 4, o // 4
                    cT = smallp.tile([128, NCH, B], dt.bfloat16, tag="cT")
                    nc.vector.tensor_mul(cT[:], eTs[g][:, :, h, :], rzT[:])
                    ysb = bigp.tile([128, NCH, D_IN, B], dt.bfloat16,
                                    tag="y", bufs=2)
                    cbc = cT.unsqueeze(2).broadcast_to(
                        [128, NCH, D_IN, B])
                    nc.vector.tensor_mul(ysb[:], xt[:], cbc)
                    k = 0
                    for ic in range(NCH):
                        for di in range(D_IN):
                            nc.tensor.matmul(
                                psY[32 * g:32 * g + 16, h, :],
                                wf[:, ic, di, 16 * o:16 * o + 16],
                                ysb[:, ic, di, :],
                                start=(k == 0), stop=(k == n_acc - 1),
                                tile_position=(0, 32 * g),
                            )
                            k += 1
                for h in range(2):
                    nc.vector.tensor_copy(sTf[:, h, :], psY[:, h, :])
                squash_from_sTf(first=False)

            # ---------- final output ----------
            for h in range(2):
                op = psp.tile([B, 128], dt.float32, tag="tiny",
                              name=f"opT{h}")
                nc.tensor.transpose(op[:], sTf[:, h, :], idf[:])
                ofin = smallp.tile([B, 128], dt.float32, tag="ofin")
                fbc = fT2[64 * h:64 * h + 64, :].unsqueeze(2).broadcast_to(
                    [B, 4, 32])
                nc.vector.tensor_mul(
                    ofin.rearrange("b (o r) -> b o r", o=4),
                    op.rearrange("b (o r) -> b o r", o=4),
                    fbc)
                nc.sync.dma_start(d_out[:, h, :], ofin[:])

    nc.compile()
    return nc


def _host_prep(x, W):
    xtc = np.ascontiguousarray(
        x.transpose(1, 2, 0).reshape(NCH, 128, D_IN, B)
        .transpose(1, 0, 2, 3).astype(bf16))
    xb = np.ascontiguousarray(
        x.transpose(0, 2, 1).reshape(B, KD).astype(bf16))
    sel = np.zeros((128, B), np.float32)
    sel[np.arange(128), np.arange(128) % 64] = 1.0
    sel = sel.astype(bf16)
    onesbd = np.zeros((128, 4), np.float32)
    for g in range(4):
        onesbd[32 * g:32 * g + 16, g] = 1.0
    idf = np.eye(128, dtype=np.float32)
    idb = np.eye(128, dtype=np.float32).astype(bf16)

    in_maps = []
    for c in range(N_CORES):
        Wc = W[c * O_LOC:(c + 1) * O_LOC]
        wfc = np.ascontiguousarray(
            Wc.transpose(1, 3, 0, 2)
            .reshape(NCH, 128, D_IN, O_LOC * D_OUT)
            .transpose(1, 0, 2, 3).astype(bf16))
        wdoc = np.zeros((4, 16, 2, KD), np.float32)
        for g in range(4):
            for h in range(2):
                o = 4 * h + g
                wdoc[g, :, h, :] = (
                    Wc[o].transpose(1, 2, 0).reshape(D_OUT, KD))
        in_maps.append({
            "xt": xtc, "wf": wfc,
            "wdo": np.ascontiguousarray(wdoc.astype(bf16)),
            "xb": xb, "sel": sel, "onesbd": onesbd,
            "identf": idf, "identb": idb,
        })
    return in_maps


def kernel(input, W):
    global last_exec_ns
    _install_ntff_hook()
    from concourse.bass_utils import run_bass_kernel_spmd

    x = np.asarray(input, dtype=np.float32)
    W = np.asarray(W, dtype=np.float32)

    if "nc" not in _CACHE:
        _CACHE["nc"] = _build()
    nc = _CACHE["nc"]

    in_maps = _host_prep(x, W)
    trace = bool(int(os.environ.get("CAPS_TRACE", "0")))
    res = run_bass_kernel_spmd(nc, in_maps, core_ids=list(range(N_CORES)),
                               trace=trace)
    last_exec_ns = res.exec_time_ns

    outs = []
    for c in range(N_CORES):
        y = res.results[c]["yout"].reshape(B, 2, 4, 32)[:, :, :, :16]
        outs.append(y.reshape(B, 8, D_OUT))
    return np.concatenate(outs, axis=1).astype(np.float32)
